# revision 6
# baseline (speedup 1.0000x reference)
"""Mean-aggregator kernel for Trainium2 (Bass/Tile), 8-core SPMD.

mailbox: [50000, 32, 128] f32  ->  out: [50000, 128] f32 = mean over axis 1.

Sharding: node axis (dim 0) split evenly across 8 cores (6250 nodes/core).
Per core the kernel is a DMA-bound streaming reduction:
  - load [128 part, 2 nodes * 32 deg * 128 feat] = 4 MB contiguous tiles
  - DVE tensor_reduce over the (innermost-permuted) deg axis
  - ACT scale by 1/32 and store [128, 256] result tiles
"""

import numpy as np

import concourse.bass as bass
import concourse.tile as tile
from concourse import bacc, mybir
from concourse.bass_utils import run_bass_kernel_spmd

N_CORES = 8
N_NODES = 50000
PER_CORE = N_NODES // N_CORES  # 6250
DEG = 32
FEAT = 128

NODES_PER_PART = 2
TILE_NODES = 128 * NODES_PER_PART  # 256
N_FULL = PER_CORE // TILE_NODES  # 24
REM = PER_CORE - N_FULL * TILE_NODES  # 106


def build():
    nc = bacc.Bacc()
    mail = nc.dram_tensor(
        "mailbox", [PER_CORE, DEG, FEAT], mybir.dt.float32, kind="ExternalInput"
    )
    out = nc.dram_tensor(
        "out", [PER_CORE, FEAT], mybir.dt.float32, kind="ExternalOutput"
    )
    scale = 1.0 / DEG

    with tile.TileContext(nc) as tc:
        with (
            tc.tile_pool(name="inp", bufs=3) as in_pool,
            tc.tile_pool(name="red", bufs=3) as red_pool,
            tc.tile_pool(name="outp", bufs=3) as out_pool,
        ):
            for i in range(N_FULL):
                n0 = i * TILE_NODES
                t = in_pool.tile([128, NODES_PER_PART * DEG * FEAT], mybir.dt.float32)
                src = mail[n0 : n0 + TILE_NODES].rearrange("(p k) d f -> p (k d f)", p=128)
                nc.gpsimd.dma_start(out=t[:], in_=src)

                red = red_pool.tile([128, NODES_PER_PART * FEAT], mybir.dt.float32)
                perm = t[:].rearrange(
                    "p (k d f) -> p k f d", k=NODES_PER_PART, d=DEG, f=FEAT
                )
                nc.vector.tensor_reduce(
                    red[:], perm, axis=mybir.AxisListType.X, op=mybir.AluOpType.add
                )

                o = out_pool.tile([128, NODES_PER_PART * FEAT], mybir.dt.float32)
                nc.scalar.mul(o[:], red[:], scale)
                dst = out[n0 : n0 + TILE_NODES].rearrange("(p k) f -> p (k f)", p=128)
                nc.scalar.dma_start(out=dst, in_=o[:])

            if REM:
                n0 = N_FULL * TILE_NODES
                t = in_pool.tile([REM, DEG * FEAT], mybir.dt.float32)
                src = mail[n0:].rearrange("p d f -> p (d f)")
                nc.gpsimd.dma_start(out=t[:], in_=src)

                red = red_pool.tile([REM, FEAT], mybir.dt.float32)
                perm = t[:].rearrange("p (d f) -> p f d", d=DEG, f=FEAT)
                nc.vector.tensor_reduce(
                    red[:], perm, axis=mybir.AxisListType.X, op=mybir.AluOpType.add
                )

                o = out_pool.tile([REM, FEAT], mybir.dt.float32)
                nc.scalar.mul(o[:], red[:], scale)
                nc.scalar.dma_start(out=out[n0:], in_=o[:])

    if not nc.is_finalized():
        nc.finalize()
    return nc


_NC_CACHE = None


def _get_nc():
    global _NC_CACHE
    if _NC_CACHE is None:
        _NC_CACHE = build()
    return _NC_CACHE


def run(mailbox: np.ndarray, trace: bool = False, **trace_kwargs):
    mailbox = np.ascontiguousarray(np.asarray(mailbox, dtype=np.float32))
    assert mailbox.shape == (N_NODES, DEG, FEAT), mailbox.shape
    nc = _get_nc()
    shards = mailbox.reshape(N_CORES, PER_CORE, DEG, FEAT)
    in_maps = [{"mailbox": shards[i]} for i in range(N_CORES)]
    res = run_bass_kernel_spmd(
        nc, in_maps, list(range(N_CORES)), trace=trace, **trace_kwargs
    )
    full = np.concatenate([res.results[i]["out"] for i in range(N_CORES)], axis=0)
    return full, res


def kernel(mailbox: np.ndarray) -> np.ndarray:
    full, _ = run(mailbox, trace=False)
    return full


# revision 9
# speedup vs baseline: 1.1814x; 1.1814x over previous
"""Mean-aggregator kernel for Trainium2 (Bass/Tile), 8-core SPMD.

mailbox: [50000, 32, 128] f32  ->  out: [50000, 128] f32 = mean over axis 1.

Sharding: node axis (dim 0) split evenly across 8 cores (6250 nodes/core).
Per core the kernel is a DMA-bound streaming reduction:
  - load [128 part, 2 nodes * 32 deg * 128 feat] = 4 MB contiguous tiles
  - DVE tensor_reduce over the (innermost-permuted) deg axis
  - ACT scale by 1/32 and store [128, 256] result tiles
"""

import numpy as np

import concourse.bass as bass
import concourse.tile as tile
from concourse import bacc, mybir
from concourse.bass_utils import run_bass_kernel_spmd

N_CORES = 8
N_NODES = 50000
PER_CORE = N_NODES // N_CORES  # 6250
DEG = 32
FEAT = 128

NODES_PER_PART = 2
TILE_NODES = 128 * NODES_PER_PART  # 256
N_FULL = PER_CORE // TILE_NODES  # 24
REM = PER_CORE - N_FULL * TILE_NODES  # 106


def build():
    nc = bacc.Bacc()
    mail = nc.dram_tensor(
        "mailbox", [PER_CORE, DEG, FEAT], mybir.dt.float32, kind="ExternalInput"
    )
    out = nc.dram_tensor(
        "out", [PER_CORE, FEAT], mybir.dt.float32, kind="ExternalOutput"
    )
    scale = 1.0 / DEG

    with tile.TileContext(nc) as tc:
        with (
            tc.tile_pool(name="inp", bufs=3) as in_pool,
            tc.tile_pool(name="outp", bufs=3) as out_pool,
        ):
            for i in range(N_FULL):
                n0 = i * TILE_NODES
                t = in_pool.tile([128, NODES_PER_PART * DEG * FEAT], mybir.dt.float32)
                src = mail[n0 : n0 + TILE_NODES].rearrange("(p k) d f -> p (k d f)", p=128)
                nc.gpsimd.dma_start(out=t[:], in_=src)

                # In-place halving tree over the deg axis: contiguous APs keep
                # DVE at 1 elem/cycle (strided-innermost tensor_reduce is ~1.6x).
                v = t[:].rearrange("p (k d f) -> p k d f", k=NODES_PER_PART, d=DEG, f=FEAT)
                h = DEG
                while h > 1:
                    h //= 2
                    nc.vector.tensor_add(
                        v[:, :, 0:h, :], v[:, :, 0:h, :], v[:, :, h : 2 * h, :]
                    )

                o = out_pool.tile([128, NODES_PER_PART * FEAT], mybir.dt.float32)
                nc.scalar.mul(
                    o[:].rearrange("p (k f) -> p k f", k=NODES_PER_PART),
                    v[:, :, 0, :],
                    scale,
                )
                dst = out[n0 : n0 + TILE_NODES].rearrange("(p k) f -> p (k f)", p=128)
                nc.scalar.dma_start(out=dst, in_=o[:])

            if REM:
                n0 = N_FULL * TILE_NODES
                t = in_pool.tile([REM, DEG * FEAT], mybir.dt.float32)
                src = mail[n0:].rearrange("p d f -> p (d f)")
                nc.gpsimd.dma_start(out=t[:], in_=src)

                v = t[:].rearrange("p (d f) -> p d f", d=DEG, f=FEAT)
                h = DEG
                while h > 1:
                    h //= 2
                    nc.vector.tensor_add(
                        v[:, 0:h, :], v[:, 0:h, :], v[:, h : 2 * h, :]
                    )

                o = out_pool.tile([REM, FEAT], mybir.dt.float32)
                nc.scalar.mul(o[:], v[:, 0, :], scale)
                nc.scalar.dma_start(out=out[n0:], in_=o[:])

    if not nc.is_finalized():
        nc.finalize()
    return nc


_NC_CACHE = None


def _get_nc():
    global _NC_CACHE
    if _NC_CACHE is None:
        _NC_CACHE = build()
    return _NC_CACHE


def run(mailbox: np.ndarray, trace: bool = False, **trace_kwargs):
    mailbox = np.ascontiguousarray(np.asarray(mailbox, dtype=np.float32))
    assert mailbox.shape == (N_NODES, DEG, FEAT), mailbox.shape
    nc = _get_nc()
    shards = mailbox.reshape(N_CORES, PER_CORE, DEG, FEAT)
    in_maps = [{"mailbox": shards[i]} for i in range(N_CORES)]
    res = run_bass_kernel_spmd(
        nc, in_maps, list(range(N_CORES)), trace=trace, **trace_kwargs
    )
    full = np.concatenate([res.results[i]["out"] for i in range(N_CORES)], axis=0)
    return full, res


def kernel(mailbox: np.ndarray) -> np.ndarray:
    full, _ = run(mailbox, trace=False)
    return full
